# revision 16
# baseline (speedup 1.0000x reference)
"""Trainium2 Bass kernel for nn_Encoder_36876589204306 (single-layer
transformer encoder: embed+posenc -> MHA -> add&LN -> FFN -> add&LN).

Sharding: pure data-parallel over batch. B=64 sequences split as 8 per
NeuronCore; every core holds the full weights, no collectives.

Per-core pipeline (N=800 tokens, D=1024, H=16 heads, depth=64, F=4096):
  - embedding gather via indirect DMA + positional-encoding add  (x f32)
  - x -> xT (PE transpose), QKV projections consuming xT in bf16
    q,k produced in "T layout" [d, n]; v produced per-batch in natural
    layout with an interleaved ones-column (stride-66-free layout) so the
    attention output matmul also produces the softmax row-sums.
  - scoresT = kT.T @ qT per (batch,head); exp on ScalarE with fused 1/8
    scale; NO max subtraction (scores are O(1) here, exact same math).
  - ctx = expT.T @ [v | 1]; divide by the row-sum column; per-head.
  - ctx -> ctxT (PE transpose), att_out = ctxT.T @ Wo + residual, LN1 (f32)
  - x1 -> x1T, h1T = relu(W1.T @ x1T + b1), out = h1T.T @ W2 + residual, LN2
All matmul operands are bf16 (fp32 accumulation in PSUM); the residual /
LayerNorm spine stays fp32.
"""

import numpy as np
import ml_dtypes

import concourse.bass as bass
import concourse.mybir as mybir
import concourse.tile as tile
from concourse import bacc
from concourse.bass import IndirectOffsetOnAxis
from concourse.bass_utils import run_bass_kernel_spmd
from concourse.masks import make_identity

# ---------------- problem dims (hardcoded per contract) ----------------
B, S, D, H, F, V = 64, 100, 1024, 16, 4096, 32000
E = D // H            # 64 head depth
NCORES = 8
BL = B // NCORES      # 8 sequences per core
N = BL * S            # 800 tokens per core
P = 128
DC = D // P           # 8 chunks of d
FC = F // P           # 32 chunks of f
EPS = 1e-6

F32 = mybir.dt.float32
BF = mybir.dt.bfloat16
AF = mybir.ActivationFunctionType
OP = mybir.AluOpType

N_CH = (N + P - 1) // P                                   # 7 token chunks
CHUNKS = [(c * P, min(P, N - c * P)) for c in range(N_CH)]
N_TILES = [(0, 512), (512, N - 512)]                      # moving-dim tiles
VG = 66   # per-head group stride in v_aug (64 v cols + 1 ones col + 1 pad)


def _bcast(ap, p=P):
    """[n] DRAM AP -> [p, n] partition-broadcast AP."""
    return bass.AP(tensor=ap.tensor, offset=ap.offset, ap=[[0, p]] + list(ap.ap))


def build_nc(flags):
    use_bq = flags["bq"]; use_bk = flags["bk"]; use_bv = flags["bv"]
    use_bo = flags["bo"]; use_b1 = flags["b1"]; use_b2 = flags["b2"]
    use_a1 = flags["a1"]; use_a2 = flags["a2"]

    nc = bacc.Bacc("TRN2", target_bir_lowering=False, debug=False,
                   num_devices=NCORES)

    tokens = nc.dram_tensor("tokens", [N], mybir.dt.int32, kind="ExternalInput").ap()
    emb = nc.dram_tensor("emb", [V, D], F32, kind="ExternalInput").ap()
    pe = nc.dram_tensor("pe", [N, D], F32, kind="ExternalInput").ap()
    wq = nc.dram_tensor("wq", [D, D], BF, kind="ExternalInput").ap()
    wk = nc.dram_tensor("wk", [D, D], BF, kind="ExternalInput").ap()
    wv = nc.dram_tensor("wv", [D, D], BF, kind="ExternalInput").ap()
    wo = nc.dram_tensor("wo", [D, D], BF, kind="ExternalInput").ap()
    w1 = nc.dram_tensor("w1", [D, F], BF, kind="ExternalInput").ap()
    w2 = nc.dram_tensor("w2", [F, D], BF, kind="ExternalInput").ap()
    bq = nc.dram_tensor("bq", [D], F32, kind="ExternalInput").ap() if use_bq else None
    bk = nc.dram_tensor("bk", [D], F32, kind="ExternalInput").ap() if use_bk else None
    bv = nc.dram_tensor("bv", [D], F32, kind="ExternalInput").ap() if use_bv else None
    bo = nc.dram_tensor("bo", [D], F32, kind="ExternalInput").ap() if use_bo else None
    b1 = nc.dram_tensor("b1", [F], F32, kind="ExternalInput").ap() if use_b1 else None
    b2 = nc.dram_tensor("b2", [D], F32, kind="ExternalInput").ap() if use_b2 else None
    g1 = nc.dram_tensor("g1", [D], F32, kind="ExternalInput").ap() if use_a1 else None
    bt1 = nc.dram_tensor("bt1", [D], F32, kind="ExternalInput").ap() if use_a1 else None
    g2 = nc.dram_tensor("g2", [D], F32, kind="ExternalInput").ap() if use_a2 else None
    bt2 = nc.dram_tensor("bt2", [D], F32, kind="ExternalInput").ap() if use_a2 else None
    out = nc.dram_tensor("out", [N, D], F32, kind="ExternalOutput").ap()

    with tile.TileContext(nc) as tc:
        # ---- whole-kernel pools ----
        cpool = tc.alloc_tile_pool(name="const", bufs=1)
        pspool = tc.alloc_tile_pool(name="ps", bufs=5, space="PSUM")
        psbpool = tc.alloc_tile_pool(name="psb", bufs=2, space="PSUM")
        spool = tc.alloc_tile_pool(name="small", bufs=8)

        idf = cpool.tile([P, P], F32, tag="idf")
        make_identity(nc, idf)
        idb = cpool.tile([P, P], BF, tag="idb")
        make_identity(nc, idb)
        epsT = cpool.tile([P, 1], F32, tag="eps")
        nc.vector.memset(epsT, EPS)

        tok = cpool.tile([P, N_CH], mybir.dt.int32, tag="tok")
        n_full = (N // P) * P
        nc.sync.dma_start(out=tok[:, 0:N // P],
                          in_=tokens[0:n_full].rearrange("(c p) -> p c", p=P))
        rem = N - n_full
        if rem:
            nc.sync.dma_start(out=tok[0:rem, N // P:N // P + 1],
                              in_=tokens[n_full:N].rearrange("(c p) -> p c", p=rem))

        # broadcast tiles for free-axis biases / affines (rarely used)
        def load_bcast(ap_, name, dt=F32, width=D):
            t = cpool.tile([P, width], dt, tag=name)
            nc.sync.dma_start(out=t, in_=_bcast(ap_))
            return t
        bvb = load_bcast(bv, "bvb") if use_bv else None
        bob = load_bcast(bo, "bob") if use_bo else None
        b2b = load_bcast(b2, "b2b") if use_b2 else None
        g1b = load_bcast(g1, "g1b") if use_a1 else None
        bt1b = load_bcast(bt1, "bt1b") if use_a1 else None
        g2b = load_bcast(g2, "g2b") if use_a2 else None
        bt2b = load_bcast(bt2, "bt2b") if use_a2 else None

        # ---- P0: embedding gather + positional encoding -> x_nat (f32) ----
        xpool = tc.alloc_tile_pool(name="xpool", bufs=1)
        x_nat = xpool.tile([P, N_CH, D], F32, tag="x_nat")
        with tc.tile_pool(name="gat", bufs=3) as gpool:
            for c, (n0, rows) in enumerate(CHUNKS):
                xg = gpool.tile([P, D], F32, tag="xg")
                nc.gpsimd.indirect_dma_start(
                    out=xg[:rows], out_offset=None, in_=emb,
                    in_offset=IndirectOffsetOnAxis(ap=tok[:rows, c:c + 1], axis=0))
                pet = gpool.tile([P, D], F32, tag="pet")
                nc.sync.dma_start(out=pet[:rows], in_=pe[n0:n0 + rows, :])
                nc.vector.tensor_add(out=x_nat[:rows, c, :], in0=xg[:rows],
                                     in1=pet[:rows])

        # ---- P1: x -> xT (bf16) ----
        bpool = tc.alloc_tile_pool(name="attn_acts", bufs=1)
        xT = bpool.tile([P, DC, N], BF, tag="xT")
        for c, (n0, rows) in enumerate(CHUNKS):
            for dq in range(2):
                ps = pspool.tile([P, 512], F32, tag="ps")
                for j in range(4):
                    d = dq * 4 + j
                    nc.tensor.transpose(out=ps[:, j * rows:(j + 1) * rows],
                                        in_=x_nat[:rows, c, d * P:(d + 1) * P],
                                        identity=idf[:rows, :rows])
                nc.vector.tensor_copy(
                    out=xT[:, dq * 4:(dq + 1) * 4, n0:n0 + rows],
                    in_=ps[:, 0:4 * rows].rearrange("p (j r) -> p j r", r=rows))

        # ---- P2: QKV projections ----
        qT = bpool.tile([P, DC, N], BF, tag="qT")
        kT = bpool.tile([P, DC, N], BF, tag="kT")
        v_aug = bpool.tile([P, BL, H * VG], BF, tag="v_aug")
        v_r = v_aug.rearrange("p b (h e) -> p b h e", e=VG)
        with tc.tile_pool(name="wqkv", bufs=2) as wpool:
            bq_s = bk_s = None
            if use_bq:
                bq_s = cpool.tile([P, DC], F32, tag="bq_s")
                nc.sync.dma_start(out=bq_s, in_=bq.rearrange("(c p) -> p c", p=P))
            if use_bk:
                bk_s = cpool.tile([P, DC], F32, tag="bk_s")
                nc.sync.dma_start(out=bk_s, in_=bk.rearrange("(c p) -> p c", p=P))

            # q, k in T layout: dst[d_col, n] (col on partitions)
            for w_dram, dst, b_s, eng in ((wq, qT, bq_s, nc.scalar),
                                          (wk, kT, bk_s, nc.vector)):
                w_s = wpool.tile([P, DC, D], BF, tag="wqkv")
                nc.sync.dma_start(out=w_s,
                                  in_=w_dram.rearrange("(c p) n -> p c n", p=P))
                for ct in range(DC):
                    for (t0, tw) in N_TILES:
                        ps = pspool.tile([P, 512], F32, tag="ps")
                        for kc in range(DC):
                            nc.tensor.matmul(ps[:, :tw],
                                             lhsT=w_s[:, kc, ct * P:(ct + 1) * P],
                                             rhs=xT[:, kc, t0:t0 + tw],
                                             start=(kc == 0), stop=(kc == DC - 1))
                        if b_s is not None:
                            nc.scalar.activation(out=dst[:, ct, t0:t0 + tw],
                                                 in_=ps[:, :tw], func=AF.Copy,
                                                 bias=b_s[:, ct:ct + 1], scale=1.0)
                        elif eng is nc.scalar:
                            nc.scalar.copy(out=dst[:, ct, t0:t0 + tw], in_=ps[:, :tw])
                        else:
                            nc.vector.tensor_copy(out=dst[:, ct, t0:t0 + tw],
                                                  in_=ps[:, :tw])

            # v in natural layout per batch, interleaved with ones columns
            wv_s = wpool.tile([P, DC, D], BF, tag="wqkv")
            nc.sync.dma_start(out=wv_s, in_=wv.rearrange("(c p) n -> p c n", p=P))
            for b in range(BL):
                for ct2 in range(2):
                    ps = pspool.tile([P, 512], F32, tag="ps")
                    for kc in range(DC):
                        nc.tensor.matmul(ps[:S, :],
                                         lhsT=xT[:, kc, b * S:(b + 1) * S],
                                         rhs=wv_s[:, kc, ct2 * 512:(ct2 + 1) * 512],
                                         start=(kc == 0), stop=(kc == DC - 1))
                    if use_bv:
                        nc.vector.tensor_add(
                            out=v_r[:S, b, ct2 * 8:(ct2 + 1) * 8, 0:64],
                            in0=ps[:S, :].rearrange("p (h e) -> p h e", e=64),
                            in1=bvb[:S, ct2 * 512:(ct2 + 1) * 512]
                                .rearrange("p (h e) -> p h e", e=64))
                    else:
                        nc.vector.tensor_copy(
                            out=v_r[:S, b, ct2 * 8:(ct2 + 1) * 8, 0:64],
                            in_=ps[:S, :].rearrange("p (h e) -> p h e", e=64))
                nc.vector.memset(v_r[:S, b, :, 64:65], 1.0)

        # ---- P3: attention ----
        expT = bpool.tile([P, H, N], BF, tag="expT")
        for h in range(H):
            pch, poff = h // 2, (h % 2) * 64
            for bq4 in range(2):
                ps = pspool.tile([P, 4, S], F32, tag="ps")
                for j in range(4):
                    b = bq4 * 4 + j
                    nc.tensor.matmul(ps[:S, j, :],
                                     lhsT=kT[poff:poff + 64, pch, b * S:(b + 1) * S],
                                     rhs=qT[poff:poff + 64, pch, b * S:(b + 1) * S],
                                     start=True, stop=True)
                nc.scalar.activation(
                    out=expT[:S, h, bq4 * 4 * S:(bq4 * 4 + 4) * S]
                        .rearrange("p (j s) -> p j s", s=S),
                    in_=ps[:S], func=AF.Exp, scale=float(1.0 / np.sqrt(E)))

        ctx_nat = bpool.tile([P, BL, D], BF, tag="ctx_nat")
        for b in range(BL):
            for hq in range(4):
                ps = pspool.tile([P, 4, VG], F32, tag="ps")
                for j in range(4):
                    h = hq * 4 + j
                    nc.tensor.matmul(ps[:S, j, 0:65],
                                     lhsT=expT[:S, h, b * S:(b + 1) * S],
                                     rhs=v_r[:S, b, h, 0:65],
                                     start=True, stop=True)
                rc = spool.tile([P, 4], F32, tag="rc")
                nc.vector.reciprocal(out=rc[:S], in_=ps[:S, :, 64])
                for j in range(4):
                    h = hq * 4 + j
                    nc.vector.tensor_scalar_mul(
                        out=ctx_nat[:S, b, h * 64:(h + 1) * 64],
                        in0=ps[:S, j, 0:64], scalar1=rc[:S, j:j + 1])

        # ---- P4: ctx -> ctxT (bf16) ----
        mpool = tc.alloc_tile_pool(name="mid", bufs=1, side="right")
        ctxT = mpool.tile([P, DC, N], BF, tag="ctxT")
        for b in range(BL):
            for dq in range(2):
                ps = psbpool.tile([P, 4, S], BF, tag="psb")
                for j in range(4):
                    d = dq * 4 + j
                    nc.tensor.transpose(out=ps[:, j, :],
                                        in_=ctx_nat[:S, b, d * P:(d + 1) * P],
                                        identity=idb[:S, :S])
                nc.vector.tensor_copy(out=ctxT[:, dq * 4:(dq + 1) * 4,
                                               b * S:(b + 1) * S],
                                      in_=ps[:, :, :])
        bpool.release()

        # ---- P5: Wo + residual -> z (f32) ----
        z = mpool.tile([P, N_CH, D], F32, tag="z")
        with tc.tile_pool(name="wo", bufs=1) as wopool:
            wo_s = wopool.tile([P, DC, D], BF, tag="wo")
            nc.sync.dma_start(out=wo_s, in_=wo.rearrange("(c p) n -> p c n", p=P))
            for c, (n0, rows) in enumerate(CHUNKS):
                for ct2 in range(2):
                    ps = pspool.tile([P, 512], F32, tag="ps")
                    for kc in range(DC):
                        nc.tensor.matmul(ps[:rows],
                                         lhsT=ctxT[:, kc, n0:n0 + rows],
                                         rhs=wo_s[:, kc, ct2 * 512:(ct2 + 1) * 512],
                                         start=(kc == 0), stop=(kc == DC - 1))
                    nc.vector.tensor_add(out=z[:rows, c, ct2 * 512:(ct2 + 1) * 512],
                                         in0=ps[:rows],
                                         in1=x_nat[:rows, c, ct2 * 512:(ct2 + 1) * 512])
                if use_bo:
                    nc.vector.tensor_add(out=z[:rows, c, :], in0=z[:rows, c, :],
                                         in1=bob[:rows])
        xpool.release()

        # ---- P6: LN1 -> x1 (f32) and x1T (bf16) ----
        x1pool = tc.alloc_tile_pool(name="x1p", bufs=1)
        f1pool = tc.alloc_tile_pool(name="f1", bufs=1)
        x1 = x1pool.tile([P, N_CH, D], F32, tag="x1")
        x1T = f1pool.tile([P, DC, N], BF, tag="x1T")

        def layer_norm(dst, src, rows, gb, bb):
            st = spool.tile([P, 2, 6], F32, tag="st")
            mv = spool.tile([P, 2], F32, tag="mv")
            nc.vector.bn_stats(out=st[:rows, 0, :], in_=src[:, 0:512])
            nc.vector.bn_stats(out=st[:rows, 1, :], in_=src[:, 512:1024])
            nc.vector.bn_aggr(out=mv[:rows], in_=st[:rows])
            nc.scalar.activation(out=mv[:rows, 1:2], in_=mv[:rows, 1:2],
                                 func=AF.Sqrt, bias=epsT[:rows], scale=1.0)
            nc.vector.reciprocal(out=mv[:rows, 1:2], in_=mv[:rows, 1:2])
            nc.vector.tensor_scalar(out=dst, in0=src,
                                    scalar1=mv[:rows, 0:1], scalar2=mv[:rows, 1:2],
                                    op0=OP.subtract, op1=OP.mult)
            if gb is not None:
                nc.vector.tensor_mul(out=dst, in0=dst, in1=gb[:rows])
                nc.vector.tensor_add(out=dst, in0=dst, in1=bb[:rows])

        for c, (n0, rows) in enumerate(CHUNKS):
            layer_norm(x1[:rows, c, :], z[:rows, c, :], rows,
                       g1b if use_a1 else None, bt1b if use_a1 else None)
            for dq in range(2):
                ps = pspool.tile([P, 512], F32, tag="ps")
                for j in range(4):
                    d = dq * 4 + j
                    nc.tensor.transpose(out=ps[:, j * rows:(j + 1) * rows],
                                        in_=x1[:rows, c, d * P:(d + 1) * P],
                                        identity=idf[:rows, :rows])
                nc.vector.tensor_copy(
                    out=x1T[:, dq * 4:(dq + 1) * 4, n0:n0 + rows],
                    in_=ps[:, 0:4 * rows].rearrange("p (j r) -> p j r", r=rows))
        mpool.release()

        # ---- P7: FFN1: h1T = relu(W1.T @ x1T + b1)  (bf16, T layout) ----
        hpool = tc.alloc_tile_pool(name="h1", bufs=1, side="right")
        h1T = hpool.tile([P, FC, N], BF, tag="h1T")
        b1_s = None
        if use_b1:
            b1_s = cpool.tile([P, FC], F32, tag="b1_s")
            nc.sync.dma_start(out=b1_s, in_=b1.rearrange("(c p) -> p c", p=P))
        w1_r = w1.rearrange("(c p) f -> p c f", p=P)
        with tc.tile_pool(name="w1s", bufs=3) as w1pool:
            for fg in range(8):
                w1t = w1pool.tile([P, DC, 512], BF, tag="w1t")
                nc.sync.dma_start(out=w1t, in_=w1_r[:, :, fg * 512:(fg + 1) * 512])
                for fc4 in range(4):
                    fabs = fg * 4 + fc4
                    for (t0, tw) in N_TILES:
                        ps = pspool.tile([P, 512], F32, tag="ps")
                        for kc in range(DC):
                            nc.tensor.matmul(ps[:, :tw],
                                             lhsT=w1t[:, kc, fc4 * P:(fc4 + 1) * P],
                                             rhs=x1T[:, kc, t0:t0 + tw],
                                             start=(kc == 0), stop=(kc == DC - 1))
                        if use_b1:
                            nc.scalar.activation(out=h1T[:, fabs, t0:t0 + tw],
                                                 in_=ps[:, :tw], func=AF.Relu,
                                                 bias=b1_s[:, fabs:fabs + 1], scale=1.0)
                        else:
                            nc.scalar.activation(out=h1T[:, fabs, t0:t0 + tw],
                                                 in_=ps[:, :tw], func=AF.Relu)
        f1pool.release()

        # ---- P8+P9: FFN2 + residual + LN2 -> out ----
        opool = tc.alloc_tile_pool(name="ostage", bufs=3)
        w2pool = tc.alloc_tile_pool(name="w2p", bufs=1)
        w2_s = w2pool.tile([P, FC, D], BF, tag="w2s")
        nc.sync.dma_start(out=w2_s, in_=w2.rearrange("(c p) n -> p c n", p=P))
        for c, (n0, rows) in enumerate(CHUNKS):
            z2t = opool.tile([P, D], F32, tag="z2")
            for ct2 in range(2):
                ps = pspool.tile([P, 512], F32, tag="ps")
                for kc in range(FC):
                    nc.tensor.matmul(ps[:rows],
                                     lhsT=h1T[:, kc, n0:n0 + rows],
                                     rhs=w2_s[:, kc, ct2 * 512:(ct2 + 1) * 512],
                                     start=(kc == 0), stop=(kc == FC - 1))
                nc.vector.tensor_add(out=z2t[:rows, ct2 * 512:(ct2 + 1) * 512],
                                     in0=ps[:rows],
                                     in1=x1[:rows, c, ct2 * 512:(ct2 + 1) * 512])
            if use_b2:
                nc.vector.tensor_add(out=z2t[:rows], in0=z2t[:rows], in1=b2b[:rows])
            ot = opool.tile([P, D], F32, tag="ot")
            layer_norm(ot[:rows], z2t[:rows], rows,
                       g2b if use_a2 else None, bt2b if use_a2 else None)
            nc.sync.dma_start(out=out[n0:n0 + rows, :], in_=ot[:rows])

        w2pool.release()
        opool.release()
        x1pool.release()
        hpool.release()
        spool.release()
        psbpool.release()
        pspool.release()
        cpool.release()

    nc.compile()
    return nc


# ---------------- host side ----------------

def _positional_encoding(seq_len, dim):
    pos = np.arange(seq_len).reshape(seq_len, 1).astype(np.float64)
    i = np.arange(dim)
    div_term = np.power(10000.0, 2 * (i // 2) / dim)
    pe = np.zeros((seq_len, dim))
    pe[:, 0::2] = np.sin(pos / div_term[0::2])
    pe[:, 1::2] = np.cos(pos / div_term[1::2])
    return pe.astype(np.float32)


_NC_CACHE = {}


def _get_nc(flags):
    key = tuple(sorted(flags.items()))
    if key not in _NC_CACHE:
        _NC_CACHE[key] = build_nc(flags)
    return _NC_CACHE[key]


def make_in_maps(tokens, emb_table, Wq, bq, Wk, bk, Wv, bv, Wo, bo,
                 W1, b1, W2, b2, gamma1, beta1, gamma2, beta2):
    bf16 = ml_dtypes.bfloat16
    f32 = np.float32

    def merge_hw(w):  # [H, D, E] -> [D, H*E]
        return np.ascontiguousarray(
            np.transpose(np.asarray(w, f32), (1, 0, 2)).reshape(D, D)).astype(bf16)

    flags = {
        "bq": bool(np.any(np.asarray(bq))), "bk": bool(np.any(np.asarray(bk))),
        "bv": bool(np.any(np.asarray(bv))), "bo": bool(np.any(np.asarray(bo))),
        "b1": bool(np.any(np.asarray(b1))), "b2": bool(np.any(np.asarray(b2))),
        "a1": not (np.all(np.asarray(gamma1) == 1.0) and not np.any(np.asarray(beta1))),
        "a2": not (np.all(np.asarray(gamma2) == 1.0) and not np.any(np.asarray(beta2))),
    }

    pe_full = np.ascontiguousarray(np.tile(_positional_encoding(S, D), (BL, 1)))
    common = {
        "emb": np.ascontiguousarray(np.asarray(emb_table, f32)),
        "pe": pe_full,
        "wq": merge_hw(Wq), "wk": merge_hw(Wk), "wv": merge_hw(Wv),
        "wo": np.ascontiguousarray(np.asarray(Wo, f32)).astype(bf16),
        "w1": np.ascontiguousarray(np.asarray(W1, f32)).astype(bf16),
        "w2": np.ascontiguousarray(np.asarray(W2, f32)).astype(bf16),
    }
    if flags["bq"]: common["bq"] = np.asarray(bq, f32).reshape(D)
    if flags["bk"]: common["bk"] = np.asarray(bk, f32).reshape(D)
    if flags["bv"]: common["bv"] = np.asarray(bv, f32).reshape(D)
    if flags["bo"]: common["bo"] = np.asarray(bo, f32).reshape(D)
    if flags["b1"]: common["b1"] = np.asarray(b1, f32).reshape(F)
    if flags["b2"]: common["b2"] = np.asarray(b2, f32).reshape(D)
    if flags["a1"]:
        common["g1"] = np.asarray(gamma1, f32).reshape(D)
        common["bt1"] = np.asarray(beta1, f32).reshape(D)
    if flags["a2"]:
        common["g2"] = np.asarray(gamma2, f32).reshape(D)
        common["bt2"] = np.asarray(beta2, f32).reshape(D)

    tokens = np.asarray(tokens, np.int32)
    in_maps = []
    for i in range(NCORES):
        m = dict(common)
        m["tokens"] = np.ascontiguousarray(
            tokens[i * BL:(i + 1) * BL].reshape(N))
        in_maps.append(m)
    return flags, in_maps


def kernel(**inputs):
    flags, in_maps = make_in_maps(**inputs)
    nc = _get_nc(flags)
    res = run_bass_kernel_spmd(nc, in_maps, list(range(NCORES)))
    outs = [np.asarray(res.results[i]["out"], np.float32).reshape(BL, S, D)
            for i in range(NCORES)]
    return np.concatenate(outs, axis=0)


# revision 21
# speedup vs baseline: 1.0868x; 1.0868x over previous
"""Trainium2 Bass kernel for nn_Encoder_36876589204306 (single-layer
transformer encoder: embed+posenc -> MHA -> add&LN -> FFN -> add&LN).

Sharding: pure data-parallel over batch. B=64 sequences split as 8 per
NeuronCore; every core holds the full weights, no collectives.

Per-core pipeline (N=800 tokens, D=1024, H=16 heads, depth=64, F=4096):
  - embedding gather via indirect DMA + positional-encoding add  (x f32)
  - x -> xT (PE transpose), QKV projections consuming xT in bf16
    q,k produced in "T layout" [d, n]; v produced per-batch in natural
    layout with an interleaved ones-column (stride-66-free layout) so the
    attention output matmul also produces the softmax row-sums.
  - scoresT = kT.T @ qT per (batch,head); exp on ScalarE with fused 1/8
    scale; NO max subtraction (scores are O(1) here, exact same math).
  - ctx = expT.T @ [v | 1]; divide by the row-sum column; per-head.
  - ctx -> ctxT (PE transpose), att_out = ctxT.T @ Wo + residual, LN1 (f32)
  - x1 -> x1T, h1T = relu(W1.T @ x1T + b1), out = h1T.T @ W2 + residual, LN2
All matmul operands are bf16 (fp32 accumulation in PSUM); the residual /
LayerNorm spine stays fp32.
"""

import numpy as np
import ml_dtypes

import concourse.bass as bass
import concourse.mybir as mybir
import concourse.tile as tile
from concourse import bacc
from concourse.bass import IndirectOffsetOnAxis
from concourse.bass_utils import run_bass_kernel_spmd
from concourse.masks import make_identity

# ---------------- problem dims (hardcoded per contract) ----------------
B, S, D, H, F, V = 64, 100, 1024, 16, 4096, 32000
E = D // H            # 64 head depth
NCORES = 8
BL = B // NCORES      # 8 sequences per core
N = BL * S            # 800 tokens per core
P = 128
DC = D // P           # 8 chunks of d
FC = F // P           # 32 chunks of f
EPS = 1e-6

F32 = mybir.dt.float32
BF = mybir.dt.bfloat16
AF = mybir.ActivationFunctionType
OP = mybir.AluOpType

N_CH = (N + P - 1) // P                                   # 7 token chunks
CHUNKS = [(c * P, min(P, N - c * P)) for c in range(N_CH)]
N_TILES = [(0, 512), (512, N - 512)]                      # moving-dim tiles
VG = 66   # per-head group stride in v_aug (64 v cols + 1 ones col + 1 pad)


def _bcast(ap, p=P):
    """[n] DRAM AP -> [p, n] partition-broadcast AP."""
    return bass.AP(tensor=ap.tensor, offset=ap.offset, ap=[[0, p]] + list(ap.ap))


def build_nc(flags):
    use_bq = flags["bq"]; use_bk = flags["bk"]; use_bv = flags["bv"]
    use_bo = flags["bo"]; use_b1 = flags["b1"]; use_b2 = flags["b2"]
    use_a1 = flags["a1"]; use_a2 = flags["a2"]

    nc = bacc.Bacc("TRN2", target_bir_lowering=False, debug=False,
                   num_devices=NCORES)

    tokens = nc.dram_tensor("tokens", [N], mybir.dt.int32, kind="ExternalInput").ap()
    emb = nc.dram_tensor("emb", [V, D], F32, kind="ExternalInput").ap()
    pe = nc.dram_tensor("pe", [N, D], F32, kind="ExternalInput").ap()
    wq = nc.dram_tensor("wq", [D, D], BF, kind="ExternalInput").ap()
    wk = nc.dram_tensor("wk", [D, D], BF, kind="ExternalInput").ap()
    wv = nc.dram_tensor("wv", [D, D], BF, kind="ExternalInput").ap()
    wo = nc.dram_tensor("wo", [D, D], BF, kind="ExternalInput").ap()
    w1 = nc.dram_tensor("w1", [D, F], BF, kind="ExternalInput").ap()
    w2 = nc.dram_tensor("w2", [F, D], BF, kind="ExternalInput").ap()
    bq = nc.dram_tensor("bq", [D], F32, kind="ExternalInput").ap() if use_bq else None
    bk = nc.dram_tensor("bk", [D], F32, kind="ExternalInput").ap() if use_bk else None
    bv = nc.dram_tensor("bv", [D], F32, kind="ExternalInput").ap() if use_bv else None
    bo = nc.dram_tensor("bo", [D], F32, kind="ExternalInput").ap() if use_bo else None
    b1 = nc.dram_tensor("b1", [F], F32, kind="ExternalInput").ap() if use_b1 else None
    b2 = nc.dram_tensor("b2", [D], F32, kind="ExternalInput").ap() if use_b2 else None
    g1 = nc.dram_tensor("g1", [D], F32, kind="ExternalInput").ap() if use_a1 else None
    bt1 = nc.dram_tensor("bt1", [D], F32, kind="ExternalInput").ap() if use_a1 else None
    g2 = nc.dram_tensor("g2", [D], F32, kind="ExternalInput").ap() if use_a2 else None
    bt2 = nc.dram_tensor("bt2", [D], F32, kind="ExternalInput").ap() if use_a2 else None
    out = nc.dram_tensor("out", [N, D], F32, kind="ExternalOutput").ap()

    with tile.TileContext(nc) as tc:
        # ---- whole-kernel pools ----
        cpool = tc.alloc_tile_pool(name="const", bufs=1)
        pspool = tc.alloc_tile_pool(name="ps", bufs=5, space="PSUM")
        psbpool = tc.alloc_tile_pool(name="psb", bufs=2, space="PSUM")
        spool = tc.alloc_tile_pool(name="small", bufs=8)

        idf = cpool.tile([P, P], F32, tag="idf")
        make_identity(nc, idf)
        idb = cpool.tile([P, P], BF, tag="idb")
        make_identity(nc, idb)
        epsT = cpool.tile([P, 1], F32, tag="eps")
        nc.vector.memset(epsT, EPS)

        tok = cpool.tile([P, N_CH], mybir.dt.int32, tag="tok")
        n_full = (N // P) * P
        nc.sync.dma_start(out=tok[:, 0:N // P],
                          in_=tokens[0:n_full].rearrange("(c p) -> p c", p=P))
        rem = N - n_full
        if rem:
            nc.sync.dma_start(out=tok[0:rem, N // P:N // P + 1],
                              in_=tokens[n_full:N].rearrange("(c p) -> p c", p=rem))

        # broadcast tiles for free-axis biases / affines (rarely used)
        def load_bcast(ap_, name, dt=F32, width=D):
            t = cpool.tile([P, width], dt, tag=name)
            nc.sync.dma_start(out=t, in_=_bcast(ap_))
            return t
        bvb = load_bcast(bv, "bvb") if use_bv else None
        bob = load_bcast(bo, "bob") if use_bo else None
        b2b = load_bcast(b2, "b2b") if use_b2 else None
        g1b = load_bcast(g1, "g1b") if use_a1 else None
        bt1b = load_bcast(bt1, "bt1b") if use_a1 else None
        g2b = load_bcast(g2, "g2b") if use_a2 else None
        bt2b = load_bcast(bt2, "bt2b") if use_a2 else None

        # ---- P0: embedding gather + positional encoding -> x_nat (f32) ----
        xpool = tc.alloc_tile_pool(name="xpool", bufs=1)
        x_nat = xpool.tile([P, N_CH, D], F32, tag="x_nat")
        with tc.tile_pool(name="gat", bufs=3) as gpool:
            for c, (n0, rows) in enumerate(CHUNKS):
                xg = gpool.tile([P, D], F32, tag="xg")
                nc.gpsimd.indirect_dma_start(
                    out=xg[:rows], out_offset=None, in_=emb,
                    in_offset=IndirectOffsetOnAxis(ap=tok[:rows, c:c + 1], axis=0))
                pet = gpool.tile([P, D], F32, tag="pet")
                nc.sync.dma_start(out=pet[:rows], in_=pe[n0:n0 + rows, :])
                nc.vector.tensor_add(out=x_nat[:rows, c, :], in0=xg[:rows],
                                     in1=pet[:rows])

        # ---- P1+P2: x -> xT (bf16) interleaved with QKV projections ----
        bpool = tc.alloc_tile_pool(name="attn_acts", bufs=1)
        xT = bpool.tile([P, DC, N], BF, tag="xT")
        qT = bpool.tile([P, DC, N], BF, tag="qT")
        kT = bpool.tile([P, DC, N], BF, tag="kT")
        v_aug = bpool.tile([P, BL, H * VG], BF, tag="v_aug")
        v_r = v_aug.rearrange("p b (h e) -> p b h e", e=VG)

        def transpose_chunk(c):
            n0, rows = CHUNKS[c]
            for dq in range(2):
                ps = pspool.tile([P, 512], F32, tag="ps")
                for j in range(4):
                    d = dq * 4 + j
                    nc.tensor.transpose(out=ps[:, j * rows:(j + 1) * rows],
                                        in_=x_nat[:rows, c, d * P:(d + 1) * P],
                                        identity=idf[:rows, :rows])
                nc.vector.tensor_copy(
                    out=xT[:, dq * 4:(dq + 1) * 4, n0:n0 + rows],
                    in_=ps[:, 0:4 * rows].rearrange("p (j r) -> p j r", r=rows))

        with tc.tile_pool(name="wqkv", bufs=3) as wpool:
            bq_s = bk_s = None
            if use_bq:
                bq_s = cpool.tile([P, DC], F32, tag="bq_s")
                nc.sync.dma_start(out=bq_s, in_=bq.rearrange("(c p) -> p c", p=P))
            if use_bk:
                bk_s = cpool.tile([P, DC], F32, tag="bk_s")
                nc.sync.dma_start(out=bk_s, in_=bk.rearrange("(c p) -> p c", p=P))
            wq_s = wpool.tile([P, DC, D], BF, tag="wqkv")
            nc.sync.dma_start(out=wq_s, in_=wq.rearrange("(c p) n -> p c n", p=P))
            wk_s = wpool.tile([P, DC, D], BF, tag="wqkv")
            nc.sync.dma_start(out=wk_s, in_=wk.rearrange("(c p) n -> p c n", p=P))
            wv_s = wpool.tile([P, DC, D], BF, tag="wqkv")
            nc.sync.dma_start(out=wv_s, in_=wv.rearrange("(c p) n -> p c n", p=P))

            def qk_tile(w_s, dst, b_s, use_act, t0, tw):
                for ct in range(DC):
                    ps = pspool.tile([P, 512], F32, tag="ps")
                    for kc in range(DC):
                        nc.tensor.matmul(ps[:, :tw],
                                         lhsT=w_s[:, kc, ct * P:(ct + 1) * P],
                                         rhs=xT[:, kc, t0:t0 + tw],
                                         start=(kc == 0), stop=(kc == DC - 1))
                    if b_s is not None:
                        nc.scalar.activation(out=dst[:, ct, t0:t0 + tw],
                                             in_=ps[:, :tw], func=AF.Copy,
                                             bias=b_s[:, ct:ct + 1], scale=1.0)
                    elif use_act:
                        nc.scalar.copy(out=dst[:, ct, t0:t0 + tw], in_=ps[:, :tw])
                    else:
                        nc.vector.tensor_copy(out=dst[:, ct, t0:t0 + tw],
                                              in_=ps[:, :tw])

            def v_batch(b):
                for ct2 in range(2):
                    ps = pspool.tile([P, 512], F32, tag="ps")
                    for kc in range(DC):
                        nc.tensor.matmul(ps[:S, :],
                                         lhsT=xT[:, kc, b * S:(b + 1) * S],
                                         rhs=wv_s[:, kc, ct2 * 512:(ct2 + 1) * 512],
                                         start=(kc == 0), stop=(kc == DC - 1))
                    if use_bv:
                        nc.vector.tensor_add(
                            out=v_r[:S, b, ct2 * 8:(ct2 + 1) * 8, 0:64],
                            in0=ps[:S, :].rearrange("p (h e) -> p h e", e=64),
                            in1=bvb[:S, ct2 * 512:(ct2 + 1) * 512]
                                .rearrange("p (h e) -> p h e", e=64))
                    else:
                        nc.vector.tensor_copy(
                            out=v_r[:S, b, ct2 * 8:(ct2 + 1) * 8, 0:64],
                            in_=ps[:S, :].rearrange("p (h e) -> p h e", e=64))
                nc.vector.memset(v_r[:S, b, :, 64:65], 1.0)

            # chunks 0-3 cover tokens 0-512 (= n-tile 0 and batches 0-4)
            for c in range(4):
                transpose_chunk(c)
            qk_tile(wq_s, qT, bq_s, True, 0, 512)
            qk_tile(wk_s, kT, bk_s, False, 0, 512)
            for b in range(4):
                v_batch(b)
            for c in range(4, N_CH):
                transpose_chunk(c)
            qk_tile(wq_s, qT, bq_s, True, 512, N - 512)
            qk_tile(wk_s, kT, bk_s, False, 512, N - 512)
            for b in range(4, BL):
                v_batch(b)

        # ---- P3: attention (software-pipelined over head-groups of 4) ----
        expT = bpool.tile([P, H, N], BF, tag="expT")
        ctx_nat = bpool.tile([P, BL, D], BF, tag="ctx_nat")

        def scores_group(hq):
            # heads 4hq .. 4hq+3 as two even/odd pairs; even head sits at
            # partition 0, odd at 64 -> distinct PE row groups, MMs overlap
            for pr in range(2):
                h0, h1 = hq * 4 + 2 * pr, hq * 4 + 2 * pr + 1
                pch = h0 // 2
                for bq4 in range(2):
                    psA = pspool.tile([P, 4, S], F32, tag="ps")
                    psB = pspool.tile([P, 4, S], F32, tag="ps")
                    for j in range(4):
                        b = bq4 * 4 + j
                        sl = slice(b * S, (b + 1) * S)
                        nc.tensor.matmul(psA[:S, j, :], lhsT=kT[0:64, pch, sl],
                                         rhs=qT[0:64, pch, sl],
                                         start=True, stop=True)
                        nc.tensor.matmul(psB[:S, j, :], lhsT=kT[64:128, pch, sl],
                                         rhs=qT[64:128, pch, sl],
                                         start=True, stop=True)
                    for h, psx in ((h0, psA), (h1, psB)):
                        nc.scalar.activation(
                            out=expT[:S, h, bq4 * 4 * S:(bq4 * 4 + 4) * S]
                                .rearrange("p (j s) -> p j s", s=S),
                            in_=psx[:S], func=AF.Exp, scale=float(1.0 / np.sqrt(E)))

        def ctx_group(hq):
            for b in range(BL):
                ps = pspool.tile([P, 4, VG], F32, tag="ps")
                for j in range(4):
                    h = hq * 4 + j
                    nc.tensor.matmul(ps[:S, j, 0:65],
                                     lhsT=expT[:S, h, b * S:(b + 1) * S],
                                     rhs=v_r[:S, b, h, 0:65],
                                     start=True, stop=True)
                rc = spool.tile([P, 4], F32, tag="rc")
                nc.vector.reciprocal(out=rc[:S], in_=ps[:S, :, 64])
                for j in range(4):
                    h = hq * 4 + j
                    nc.vector.tensor_scalar_mul(
                        out=ctx_nat[:S, b, h * 64:(h + 1) * 64],
                        in0=ps[:S, j, 0:64], scalar1=rc[:S, j:j + 1])

        # pipeline: scores of group hq+1 issue on PE while ACT exps group hq
        scores_group(0)
        for hq in range(4):
            if hq + 1 < 4:
                scores_group(hq + 1)
            ctx_group(hq)

        # ---- P4: ctx -> ctxT (bf16) ----
        mpool = tc.alloc_tile_pool(name="mid", bufs=1, side="right")
        ctxT = mpool.tile([P, DC, N], BF, tag="ctxT")
        for b in range(BL):
            for dq in range(2):
                ps = psbpool.tile([P, 4, S], BF, tag="psb")
                for j in range(4):
                    d = dq * 4 + j
                    nc.tensor.transpose(out=ps[:, j, :],
                                        in_=ctx_nat[:S, b, d * P:(d + 1) * P],
                                        identity=idb[:S, :S])
                nc.vector.tensor_copy(out=ctxT[:, dq * 4:(dq + 1) * 4,
                                               b * S:(b + 1) * S],
                                      in_=ps[:, :, :])
        bpool.release()

        # ---- P5: Wo + residual -> z (f32) ----
        z = mpool.tile([P, N_CH, D], F32, tag="z")
        with tc.tile_pool(name="wo", bufs=1) as wopool:
            wo_s = wopool.tile([P, DC, D], BF, tag="wo")
            nc.sync.dma_start(out=wo_s, in_=wo.rearrange("(c p) n -> p c n", p=P))
            for c, (n0, rows) in enumerate(CHUNKS):
                for ct2 in range(2):
                    ps = pspool.tile([P, 512], F32, tag="ps")
                    for kc in range(DC):
                        nc.tensor.matmul(ps[:rows],
                                         lhsT=ctxT[:, kc, n0:n0 + rows],
                                         rhs=wo_s[:, kc, ct2 * 512:(ct2 + 1) * 512],
                                         start=(kc == 0), stop=(kc == DC - 1))
                    nc.vector.tensor_add(out=z[:rows, c, ct2 * 512:(ct2 + 1) * 512],
                                         in0=ps[:rows],
                                         in1=x_nat[:rows, c, ct2 * 512:(ct2 + 1) * 512])
                if use_bo:
                    nc.vector.tensor_add(out=z[:rows, c, :], in0=z[:rows, c, :],
                                         in1=bob[:rows])
        xpool.release()

        # ---- P6: LN1 -> x1 (f32) and x1T (bf16) ----
        x1pool = tc.alloc_tile_pool(name="x1p", bufs=1)
        # prefetch W2 now: the 8MB DMA overlaps LN1 + FFN1 compute
        w2pool = tc.alloc_tile_pool(name="w2p", bufs=1)
        w2_s = w2pool.tile([P, FC, D], BF, tag="w2s")
        nc.sync.dma_start(out=w2_s, in_=w2.rearrange("(c p) n -> p c n", p=P))
        f1pool = tc.alloc_tile_pool(name="f1", bufs=1)
        x1 = x1pool.tile([P, N_CH, D], F32, tag="x1")
        x1T = f1pool.tile([P, DC, N], BF, tag="x1T")

        def layer_norm(dst, src, rows, gb, bb):
            st = spool.tile([P, 2, 6], F32, tag="st")
            mv = spool.tile([P, 2], F32, tag="mv")
            nc.vector.bn_stats(out=st[:rows, 0, :], in_=src[:, 0:512])
            nc.vector.bn_stats(out=st[:rows, 1, :], in_=src[:, 512:1024])
            nc.vector.bn_aggr(out=mv[:rows], in_=st[:rows])
            nc.scalar.activation(out=mv[:rows, 1:2], in_=mv[:rows, 1:2],
                                 func=AF.Sqrt, bias=epsT[:rows], scale=1.0)
            nc.vector.reciprocal(out=mv[:rows, 1:2], in_=mv[:rows, 1:2])
            nc.vector.tensor_scalar(out=dst, in0=src,
                                    scalar1=mv[:rows, 0:1], scalar2=mv[:rows, 1:2],
                                    op0=OP.subtract, op1=OP.mult)
            if gb is not None:
                nc.vector.tensor_mul(out=dst, in0=dst, in1=gb[:rows])
                nc.vector.tensor_add(out=dst, in0=dst, in1=bb[:rows])

        for c, (n0, rows) in enumerate(CHUNKS):
            layer_norm(x1[:rows, c, :], z[:rows, c, :], rows,
                       g1b if use_a1 else None, bt1b if use_a1 else None)
            for dq in range(2):
                ps = pspool.tile([P, 512], F32, tag="ps")
                for j in range(4):
                    d = dq * 4 + j
                    nc.tensor.transpose(out=ps[:, j * rows:(j + 1) * rows],
                                        in_=x1[:rows, c, d * P:(d + 1) * P],
                                        identity=idf[:rows, :rows])
                nc.vector.tensor_copy(
                    out=x1T[:, dq * 4:(dq + 1) * 4, n0:n0 + rows],
                    in_=ps[:, 0:4 * rows].rearrange("p (j r) -> p j r", r=rows))
        mpool.release()

        # ---- P7: FFN1: h1T = relu(W1.T @ x1T + b1)  (bf16, T layout) ----
        hpool = tc.alloc_tile_pool(name="h1", bufs=1, side="right")
        h1T = hpool.tile([P, FC, N], BF, tag="h1T")
        b1_s = None
        if use_b1:
            b1_s = cpool.tile([P, FC], F32, tag="b1_s")
            nc.sync.dma_start(out=b1_s, in_=b1.rearrange("(c p) -> p c", p=P))
        w1_r = w1.rearrange("(c p) f -> p c f", p=P)
        with tc.tile_pool(name="w1s", bufs=3) as w1pool:
            for fg in range(8):
                w1t = w1pool.tile([P, DC, 512], BF, tag="w1t")
                nc.sync.dma_start(out=w1t, in_=w1_r[:, :, fg * 512:(fg + 1) * 512])
                for fc4 in range(4):
                    fabs = fg * 4 + fc4
                    for (t0, tw) in N_TILES:
                        ps = pspool.tile([P, 512], F32, tag="ps")
                        for kc in range(DC):
                            nc.tensor.matmul(ps[:, :tw],
                                             lhsT=w1t[:, kc, fc4 * P:(fc4 + 1) * P],
                                             rhs=x1T[:, kc, t0:t0 + tw],
                                             start=(kc == 0), stop=(kc == DC - 1))
                        if use_b1:
                            nc.scalar.activation(out=h1T[:, fabs, t0:t0 + tw],
                                                 in_=ps[:, :tw], func=AF.Relu,
                                                 bias=b1_s[:, fabs:fabs + 1], scale=1.0)
                        else:
                            nc.scalar.activation(out=h1T[:, fabs, t0:t0 + tw],
                                                 in_=ps[:, :tw], func=AF.Relu)
        f1pool.release()

        # ---- P8+P9: FFN2 + residual + LN2 -> out ----
        opool = tc.alloc_tile_pool(name="ostage", bufs=3)
        for c, (n0, rows) in enumerate(CHUNKS):
            z2t = opool.tile([P, D], F32, tag="z2")
            for ct2 in range(2):
                ps = pspool.tile([P, 512], F32, tag="ps")
                for kc in range(FC):
                    nc.tensor.matmul(ps[:rows],
                                     lhsT=h1T[:, kc, n0:n0 + rows],
                                     rhs=w2_s[:, kc, ct2 * 512:(ct2 + 1) * 512],
                                     start=(kc == 0), stop=(kc == FC - 1))
                nc.vector.tensor_add(out=z2t[:rows, ct2 * 512:(ct2 + 1) * 512],
                                     in0=ps[:rows],
                                     in1=x1[:rows, c, ct2 * 512:(ct2 + 1) * 512])
            if use_b2:
                nc.vector.tensor_add(out=z2t[:rows], in0=z2t[:rows], in1=b2b[:rows])
            ot = opool.tile([P, D], F32, tag="ot")
            layer_norm(ot[:rows], z2t[:rows], rows,
                       g2b if use_a2 else None, bt2b if use_a2 else None)
            nc.sync.dma_start(out=out[n0:n0 + rows, :], in_=ot[:rows])

        opool.release()
        w2pool.release()
        x1pool.release()
        hpool.release()
        spool.release()
        psbpool.release()
        pspool.release()
        cpool.release()

    nc.compile()
    return nc


# ---------------- host side ----------------

def _positional_encoding(seq_len, dim):
    pos = np.arange(seq_len).reshape(seq_len, 1).astype(np.float64)
    i = np.arange(dim)
    div_term = np.power(10000.0, 2 * (i // 2) / dim)
    pe = np.zeros((seq_len, dim))
    pe[:, 0::2] = np.sin(pos / div_term[0::2])
    pe[:, 1::2] = np.cos(pos / div_term[1::2])
    return pe.astype(np.float32)


_NC_CACHE = {}


def _get_nc(flags):
    key = tuple(sorted(flags.items()))
    if key not in _NC_CACHE:
        _NC_CACHE[key] = build_nc(flags)
    return _NC_CACHE[key]


def make_in_maps(tokens, emb_table, Wq, bq, Wk, bk, Wv, bv, Wo, bo,
                 W1, b1, W2, b2, gamma1, beta1, gamma2, beta2):
    bf16 = ml_dtypes.bfloat16
    f32 = np.float32

    def merge_hw(w):  # [H, D, E] -> [D, H*E]
        return np.ascontiguousarray(
            np.transpose(np.asarray(w, f32), (1, 0, 2)).reshape(D, D)).astype(bf16)

    flags = {
        "bq": bool(np.any(np.asarray(bq))), "bk": bool(np.any(np.asarray(bk))),
        "bv": bool(np.any(np.asarray(bv))), "bo": bool(np.any(np.asarray(bo))),
        "b1": bool(np.any(np.asarray(b1))), "b2": bool(np.any(np.asarray(b2))),
        "a1": not (np.all(np.asarray(gamma1) == 1.0) and not np.any(np.asarray(beta1))),
        "a2": not (np.all(np.asarray(gamma2) == 1.0) and not np.any(np.asarray(beta2))),
    }

    pe_full = np.ascontiguousarray(np.tile(_positional_encoding(S, D), (BL, 1)))
    common = {
        "emb": np.ascontiguousarray(np.asarray(emb_table, f32)),
        "pe": pe_full,
        "wq": merge_hw(Wq), "wk": merge_hw(Wk), "wv": merge_hw(Wv),
        "wo": np.ascontiguousarray(np.asarray(Wo, f32)).astype(bf16),
        "w1": np.ascontiguousarray(np.asarray(W1, f32)).astype(bf16),
        "w2": np.ascontiguousarray(np.asarray(W2, f32)).astype(bf16),
    }
    if flags["bq"]: common["bq"] = np.asarray(bq, f32).reshape(D)
    if flags["bk"]: common["bk"] = np.asarray(bk, f32).reshape(D)
    if flags["bv"]: common["bv"] = np.asarray(bv, f32).reshape(D)
    if flags["bo"]: common["bo"] = np.asarray(bo, f32).reshape(D)
    if flags["b1"]: common["b1"] = np.asarray(b1, f32).reshape(F)
    if flags["b2"]: common["b2"] = np.asarray(b2, f32).reshape(D)
    if flags["a1"]:
        common["g1"] = np.asarray(gamma1, f32).reshape(D)
        common["bt1"] = np.asarray(beta1, f32).reshape(D)
    if flags["a2"]:
        common["g2"] = np.asarray(gamma2, f32).reshape(D)
        common["bt2"] = np.asarray(beta2, f32).reshape(D)

    tokens = np.asarray(tokens, np.int32)
    in_maps = []
    for i in range(NCORES):
        m = dict(common)
        m["tokens"] = np.ascontiguousarray(
            tokens[i * BL:(i + 1) * BL].reshape(N))
        in_maps.append(m)
    return flags, in_maps


def kernel(**inputs):
    flags, in_maps = make_in_maps(**inputs)
    nc = _get_nc(flags)
    res = run_bass_kernel_spmd(nc, in_maps, list(range(NCORES)))
    outs = [np.asarray(res.results[i]["out"], np.float32).reshape(BL, S, D)
            for i in range(NCORES)]
    return np.concatenate(outs, axis=0)
